# revision 1
# baseline (speedup 1.0000x reference)
"""Depthwise 3D Gaussian conv (5x5x5, SAME) on x[2,16,64,128,128] f32.

Strategy: the Gaussian kernel is rank-1 separable (k3 = g (x) g (x) g), so the
3D conv factors into three 1D 5-tap convs along D, H, W.  Each 1D conv along
the SBUF *partition* axis is a banded-matrix matmul on the PE.  We exploit
"data-as-weights" matmuls (out = X_tile^T @ B) to fuse the axis rotation
(partition <-> tile-column swap) into the conv itself:

  per 2-volume pack (partitions = (v,d), free = (h,w)):
    stage1: out[w, vd'] = sum_vd X[vd, (h,w)] * BD[vd, vd']   (conv-D + rot)
    stage2: out[h, w']  = sum_w X3[w, (h,vd)] * BW[w, w']     (conv-W + rot)
    stage3: out[h', n]  = sum_h BH[h, h'] * X5[h, (vd,w)]     (conv-H, banded)

Sharding: 32 (n,c) volumes -> 8 cores x 4 volumes (2 packs of 2).  Depthwise
conv has no cross-channel/batch mixing so no halo exchange is needed.

Compute dtype: fp16 operands with fp32 PSUM accumulation (error ~5e-4 rel).
"""

import numpy as np

N_CORES = 8
D, H, W = 64, 128, 128
HW = H * W  # 16384
PACKS = 2  # per core; each pack = 2 volumes stacked on partitions (2*64=128)
CHUNK = 2048  # DMA / copy chunk (free-dim elems)

_compiled = None  # (nc, run) cache so repeated kernel() calls reuse the build


def _taps_from_weight(weight):
    """Extract per-axis 5-tap filters (center-normalized) + global scale."""
    k3 = np.asarray(weight, np.float64)[0, 0]  # [5,5,5]
    c = k3[2, 2, 2]
    td = k3[:, 2, 2] / c
    th = k3[2, :, 2] / c
    tw = k3[2, 2, :] / c
    return td, th, tw, c


def _banded(taps, n):
    """B[i, j] = taps[i-j+2] for |i-j|<=2 (SAME zero padding via truncation)."""
    B = np.zeros((n, n), np.float64)
    for i in range(n):
        for j in range(max(0, i - 2), min(n, i + 3)):
            B[i, j] = taps[i - j + 2]
    return B


def _build_mats(weight):
    td, th, tw, c = _taps_from_weight(weight)
    bd64 = _banded(td, 64)
    BD = np.zeros((128, 128), np.float64)
    BD[:64, :64] = bd64
    BD[64:, 64:] = bd64  # block-diag: two volumes per pack, no cross mixing
    BW = _banded(tw, 128)
    BH = _banded(th, 128) * c  # fold the global scale into the last pass
    f16 = np.float16
    return BD.astype(f16), BW.astype(f16), BH.astype(f16)


def _build_program(n_loop=1):
    import concourse.mybir as mybir
    from concourse import bacc, tile

    f32 = mybir.dt.float32
    f16 = mybir.dt.float16

    nc = bacc.Bacc(None)
    xin = nc.declare_dram_parameter("xin", [PACKS, 128, HW], f32, isOutput=False)
    bd = nc.declare_dram_parameter("bd", [128, 128], f16, isOutput=False)
    bw = nc.declare_dram_parameter("bw", [128, 128], f16, isOutput=False)
    bh = nc.declare_dram_parameter("bh", [128, 128], f16, isOutput=False)
    yout = nc.declare_dram_parameter("yout", [PACKS, 128, HW], f32, isOutput=True)

    with tile.TileContext(nc) as tc:
        with (
            tc.tile_pool(name="wts", bufs=1) as wts,
            tc.tile_pool(name="ld", bufs=3) as ldp,
            tc.tile_pool(name="x16p", bufs=2) as x16p,
            tc.tile_pool(name="big", bufs=1) as big,
            tc.tile_pool(name="ps", bufs=4, space="PSUM") as psp,
            tc.tile_pool(name="st", bufs=3) as stp,
        ):
            BDt = wts.tile([128, 128], f16, tag="bd")
            BWt = wts.tile([128, 128], f16, tag="bw")
            BHt = wts.tile([128, 128], f16, tag="bh")
            nc.sync.dma_start(BDt[:], bd[:])
            nc.sync.dma_start(BWt[:], bw[:])
            nc.sync.dma_start(BHt[:], bh[:])

            for p in [pp for _ in range(n_loop) for pp in range(PACKS)]:
                x16 = x16p.tile([128, HW], f16, tag="x16")
                X3 = big.tile([128, HW], f16, tag="X3")
                X5 = big.tile([128, HW], f16, tag="X5")

                # ---- load + cast f32 -> f16 (DVE) ----
                for ci in range(HW // CHUNK):
                    ld = ldp.tile([128, CHUNK], f32, tag="ld")
                    sl = slice(ci * CHUNK, (ci + 1) * CHUNK)
                    nc.sync.dma_start(ld[:], xin[p, :, sl])
                    nc.vector.tensor_copy(x16[:, sl], ld[:])

                # ---- stage 1: conv-D (+ rotate w onto partitions) ----
                # lhsT = x16[:, h*128:(h+1)*128]  (partitions=vd, cols=w)
                # out[w, vd'] -> X3[w, h*128 + vd']  (free = (h, vd))
                for g in range(16):
                    ps = psp.tile([128, 1024], f32, tag="ps")
                    for hh in range(8):
                        h = g * 8 + hh
                        nc.tensor.matmul(
                            ps[:, hh * 128 : (hh + 1) * 128],
                            lhsT=x16[:, h * 128 : (h + 1) * 128],
                            rhs=BDt[:],
                        )
                    if g % 2 == 0:
                        nc.scalar.copy(X3[:, g * 1024 : (g + 1) * 1024], ps[:])
                    else:
                        nc.vector.tensor_copy(X3[:, g * 1024 : (g + 1) * 1024], ps[:])

                # ---- stage 2: conv-W (+ rotate h onto partitions) ----
                # lhsT = X3[:, vd::128]  (partitions=w, cols=h, strided)
                # out[h, w'] -> X5[h, vd*128 + w']  (free = (vd, w))
                X3r = X3[:].rearrange("p (h v) -> p v h", v=128)
                for g in range(16):
                    ps = psp.tile([128, 1024], f32, tag="ps")
                    for vv in range(8):
                        vd = g * 8 + vv
                        nc.tensor.matmul(
                            ps[:, vv * 128 : (vv + 1) * 128],
                            lhsT=X3r[:, vd, :],
                            rhs=BWt[:],
                        )
                    if g % 2 == 0:
                        nc.vector.tensor_copy(X5[:, g * 1024 : (g + 1) * 1024], ps[:])
                    else:
                        nc.scalar.copy(X5[:, g * 1024 : (g + 1) * 1024], ps[:])

                # ---- stage 3: conv-H (banded, stationary BH) ----
                # out[h', n] = sum_h BH[h, h'] X5[h, n]; DMA PSUM -> DRAM
                for g in range(16):
                    ps = psp.tile([128, 1024], f32, tag="ps")
                    base = g * 1024
                    nc.tensor.matmul(
                        ps[:, 0:512], lhsT=BHt[:], rhs=X5[:, base : base + 512]
                    )
                    nc.tensor.matmul(
                        ps[:, 512:1024],
                        lhsT=BHt[:],
                        rhs=X5[:, base + 512 : base + 1024],
                    )
                    st = stp.tile([128, 1024], f32, tag="st")
                    if g % 2 == 0:
                        nc.scalar.copy(st[:], ps[:])
                    else:
                        nc.vector.tensor_copy(st[:], ps[:])
                    nc.sync.dma_start(yout[p, :, base : base + 1024], st[:])
    nc.finalize()
    return nc


def _shard_inputs(x, weight):
    BD, BW, BH = _build_mats(weight)
    xs = np.ascontiguousarray(x, np.float32).reshape(32, D, H, W)
    in_maps = []
    for k in range(N_CORES):
        core_x = xs[4 * k : 4 * k + 4].reshape(PACKS, 128, HW)
        in_maps.append(
            {
                "xin": np.ascontiguousarray(core_x),
                "bd": BD,
                "bw": BW,
                "bh": BH,
            }
        )
    return in_maps


def _unshard(results):
    # yout[p, h, vd*128 + w] holds out[vol=2p+v, d, h, w]
    vols = np.empty((32, D, H, W), np.float32)
    for k in range(N_CORES):
        y = results[k]["yout"].reshape(PACKS, H, 2, D, W)  # [p, h, v, d, w]
        vols[4 * k : 4 * k + 4] = y.transpose(0, 2, 3, 1, 4).reshape(4, D, H, W)
    return vols.reshape(2, 16, D, H, W)


def kernel(x, weight):
    global _compiled
    from concourse.bass_utils import run_bass_kernel_spmd

    if _compiled is None:
        _compiled = _build_program()
    nc = _compiled
    in_maps = _shard_inputs(x, weight)
    res = run_bass_kernel_spmd(nc, in_maps, list(range(N_CORES)))
    return _unshard(res.results)



# revision 2
# speedup vs baseline: 1.0380x; 1.0380x over previous
"""Depthwise 3D Gaussian conv (5x5x5, SAME) on x[2,16,64,128,128] f32.

Strategy: the Gaussian kernel is rank-1 separable (k3 = g (x) g (x) g), so the
3D conv factors into three 1D 5-tap convs along D, W, H.  Each 1D conv along
the SBUF *partition* axis is a banded-matrix matmul on the PE.  We exploit
"data-as-weights" matmuls (out = X_tile^T @ B) to fuse the axis rotation
(partition <-> tile-column swap) into the conv itself:

  per 2-volume pack (partitions = (v,d), free = (h,w)):
    stage1: out[w, vd'] = sum_vd X[vd, (h,w)] * BD[vd, vd']   (conv-D + rot)
    stage2: out[h, w']  = sum_w X3[w, (vd,h)] * BW[w, w']     (conv-W + rot)
    stage3: out[h', n]  = sum_h BH[h, h'] * X5[h, (vd,w)]     (conv-H, banded)

Perf notes (vs the f32 baseline):
  - HBM I/O is f16 both ways (host casts f32<->f16); halves DMA time.
  - stage1's PSUM evacuation scatters to a (vd, h) free layout.  The copy is
    1x-mode regardless (f32 PSUM source), so the scatter is free, and it
    makes stage2's LDWEIGHTS columns contiguous (FWL-eligible) instead of
    stride-128 (which measured ~256ns vs ~150ns per load).
  - single-bank [128,512] PSUM tiles, 8 bufs, evacuations alternate
    DVE/ACT, so the PE is never gated on a slow drain.

Sharding: 32 (n,c) volumes -> 8 cores x 4 volumes (2 packs of 2).  Depthwise
conv has no cross-channel/batch mixing so no halo exchange is needed.
"""

import numpy as np

N_CORES = 8
D, H, W = 64, 128, 128
HW = H * W  # 16384
PACKS = 2  # per core; each pack = 2 volumes stacked on partitions (2*64=128)
LCHUNK = 2048  # load DMA chunk (free-dim elems)

_compiled = None  # (nc, run) cache so repeated kernel() calls reuse the build


def _taps_from_weight(weight):
    """Extract per-axis 5-tap filters (center-normalized) + global scale."""
    k3 = np.asarray(weight, np.float64)[0, 0]  # [5,5,5]
    c = k3[2, 2, 2]
    td = k3[:, 2, 2] / c
    th = k3[2, :, 2] / c
    tw = k3[2, 2, :] / c
    return td, th, tw, c


def _banded(taps, n):
    """B[i, j] = taps[i-j+2] for |i-j|<=2 (SAME zero padding via truncation)."""
    B = np.zeros((n, n), np.float64)
    for i in range(n):
        for j in range(max(0, i - 2), min(n, i + 3)):
            B[i, j] = taps[i - j + 2]
    return B


def _build_mats(weight):
    td, th, tw, c = _taps_from_weight(weight)
    bd64 = _banded(td, 64)
    BD = np.zeros((128, 128), np.float64)
    BD[:64, :64] = bd64
    BD[64:, 64:] = bd64  # block-diag: two volumes per pack, no cross mixing
    BW = _banded(tw, 128)
    BH = _banded(th, 128) * c  # fold the global scale into the last pass
    f16 = np.float16
    return BD.astype(f16), BW.astype(f16), BH.astype(f16)


def _build_program():
    import concourse.mybir as mybir
    from concourse import bacc, tile

    f32 = mybir.dt.float32
    f16 = mybir.dt.float16

    nc = bacc.Bacc(None)
    xin = nc.declare_dram_parameter("xin", [PACKS, 128, HW], f16, isOutput=False)
    bd = nc.declare_dram_parameter("bd", [128, 128], f16, isOutput=False)
    bw = nc.declare_dram_parameter("bw", [128, 128], f16, isOutput=False)
    bh = nc.declare_dram_parameter("bh", [128, 128], f16, isOutput=False)
    yout = nc.declare_dram_parameter("yout", [PACKS, 128, HW], f16, isOutput=True)

    with tile.TileContext(nc) as tc:
        with (
            tc.tile_pool(name="wts", bufs=1) as wts,
            tc.tile_pool(name="x16p", bufs=2) as x16p,
            tc.tile_pool(name="x3p", bufs=1) as x3p,
            tc.tile_pool(name="x5p", bufs=1) as x5p,
            tc.tile_pool(name="ps", bufs=8, space="PSUM") as psp,
            tc.tile_pool(name="st", bufs=4) as stp,
        ):
            BDt = wts.tile([128, 128], f16, tag="bd")
            BWt = wts.tile([128, 128], f16, tag="bw")
            BHt = wts.tile([128, 128], f16, tag="bh")
            nc.sync.dma_start(BDt[:], bd[:])
            nc.sync.dma_start(BWt[:], bw[:])
            nc.sync.dma_start(BHt[:], bh[:])

            def evac(t, dst, src):
                # alternate PSUM evacuations between DVE and ACT
                if t % 2 == 0:
                    nc.vector.tensor_copy(dst, src)
                else:
                    nc.scalar.copy(dst, src)

            for p in range(PACKS):
                x16 = x16p.tile([128, HW], f16, tag="x16")
                X3T = x3p.tile([128, HW], f16, tag="x3t")
                X5 = x5p.tile([128, HW], f16, tag="x5")

                # ---- load (already f16 on host) ----
                for ci in range(HW // LCHUNK):
                    sl = slice(ci * LCHUNK, (ci + 1) * LCHUNK)
                    nc.sync.dma_start(x16[:, sl], xin[p, :, sl])

                # ---- stage 1: conv-D (+ rotate w onto partitions) ----
                # lhsT = x16[:, h*128:(h+1)*128]  (partitions=vd, cols=w)
                # out[w, vd'] scattered -> X3T[w, vd'*128 + h]  (free = (vd, h))
                X3Tr = X3T[:].rearrange("p (v h) -> p h v", h=128)
                for t in range(32):
                    ps = psp.tile([128, 512], f32, tag="ps")
                    for hh in range(4):
                        h = 4 * t + hh
                        nc.tensor.matmul(
                            ps[:, hh * 128 : (hh + 1) * 128],
                            lhsT=x16[:, h * 128 : (h + 1) * 128],
                            rhs=BDt[:],
                        )
                    evac(
                        t,
                        X3Tr[:, 4 * t : 4 * t + 4, :],
                        ps[:].rearrange("p (h v) -> p h v", v=128),
                    )

                # ---- stage 2: conv-W (+ rotate h onto partitions) ----
                # lhsT = X3T[:, vd*128:(vd+1)*128]  (partitions=w, cols=h,
                # contiguous thanks to stage1's scattered store)
                # out[h, w'] -> X5[h, vd*128 + w']  (free = (vd, w))
                for t in range(32):
                    ps = psp.tile([128, 512], f32, tag="ps")
                    for vv in range(4):
                        vd = 4 * t + vv
                        nc.tensor.matmul(
                            ps[:, vv * 128 : (vv + 1) * 128],
                            lhsT=X3T[:, vd * 128 : (vd + 1) * 128],
                            rhs=BWt[:],
                        )
                    evac(t, X5[:, t * 512 : (t + 1) * 512], ps[:])

                # ---- stage 3: conv-H (banded, stationary BH) ----
                # out[h', n] = sum_h BH[h, h'] X5[h, n]; cast f16, DMA out
                for t in range(16):
                    st = stp.tile([128, 1024], f16, tag="st")
                    for u in range(2):
                        c = 2 * t + u
                        ps = psp.tile([128, 512], f32, tag="ps")
                        nc.tensor.matmul(
                            ps[:],
                            lhsT=BHt[:],
                            rhs=X5[:, c * 512 : (c + 1) * 512],
                        )
                        evac(c, st[:, u * 512 : (u + 1) * 512], ps[:])
                    nc.sync.dma_start(yout[p, :, t * 1024 : (t + 1) * 1024], st[:])
    nc.finalize()
    return nc


def _shard_inputs(x, weight):
    BD, BW, BH = _build_mats(weight)
    xs = np.asarray(x, np.float32).astype(np.float16).reshape(32, D, H, W)
    in_maps = []
    for k in range(N_CORES):
        core_x = xs[4 * k : 4 * k + 4].reshape(PACKS, 128, HW)
        in_maps.append(
            {
                "xin": np.ascontiguousarray(core_x),
                "bd": BD,
                "bw": BW,
                "bh": BH,
            }
        )
    return in_maps


def _unshard(results):
    # yout[p, h, vd*128 + w] holds out[vol=2p+v, d, h, w]  (f16 -> f32)
    vols = np.empty((32, D, H, W), np.float32)
    for k in range(N_CORES):
        y = results[k]["yout"].astype(np.float32).reshape(PACKS, H, 2, D, W)
        vols[4 * k : 4 * k + 4] = y.transpose(0, 2, 3, 1, 4).reshape(4, D, H, W)
    return vols.reshape(2, 16, D, H, W)


def kernel(x, weight):
    global _compiled
    from concourse.bass_utils import run_bass_kernel_spmd

    if _compiled is None:
        _compiled = _build_program()
    nc = _compiled
    in_maps = _shard_inputs(x, weight)
    res = run_bass_kernel_spmd(nc, in_maps, list(range(N_CORES)))
    return _unshard(res.results)


# revision 5
# speedup vs baseline: 2.6355x; 2.5391x over previous
"""Depthwise 3D Gaussian conv (5x5x5, SAME) on x[2,16,64,128,128] f32.

Strategy: the Gaussian kernel is rank-1 separable (k3 = g (x) g (x) g), so the
3D conv factors into three 1D 5-tap convs along D, W, H.  Each 1D conv along
the SBUF *partition* axis is a banded-matrix matmul on the PE.  We exploit
"data-as-weights" matmuls (out = X_tile^T @ B) to fuse the axis rotation
(partition <-> tile-column swap) into the conv itself:

  per 2-volume pack (partitions = (v,d), free = (h,w)):
    stage1: out[w, vd'] = sum_vd X[vd, (h,w)] * BD[vd, vd']   (conv-D + rot)
    stage2: out[h, w']  = sum_w X3[w, (vd,h)] * BW[w, w']     (conv-W + rot)
    stage3: out[h', n]  = sum_h BH[h, h'] * X5[h, (vd,w)]     (conv-H, banded)

Perf notes (vs the f32 baseline):
  - HBM I/O is f16 both ways (host casts f32<->f16); halves DMA time.
  - stage1's PSUM evacuation scatters to a (vd, h) free layout, which makes
    stage2's LDWEIGHTS columns contiguous (measured 56ns/pair vs 121ns for
    stride-128 weight loads).  The copy iterates (vd, h8) so the strided
    side is the PSUM *read* (free: 1223ns, same as a natural copy) and the
    SBUF writes land in 16B-contiguous runs -- dst-strided scatter measured
    4708ns, 3.9x worse.
  - [128,1024] PSUM tiles (2 banks), 4 bufs, evacuations alternate DVE/ACT,
    so the PE is never gated on a slow drain.

Sharding: 32 (n,c) volumes -> 8 cores x 4 volumes (2 packs of 2).  Depthwise
conv has no cross-channel/batch mixing so no halo exchange is needed.
"""

import numpy as np

N_CORES = 8
D, H, W = 64, 128, 128
HW = H * W  # 16384
PACKS = 2  # per core; each pack = 2 volumes stacked on partitions (2*64=128)
LCHUNK = 2048  # load DMA chunk (free-dim elems)

_compiled = None  # (nc, run) cache so repeated kernel() calls reuse the build


def _taps_from_weight(weight):
    """Extract per-axis 5-tap filters (center-normalized) + global scale."""
    k3 = np.asarray(weight, np.float64)[0, 0]  # [5,5,5]
    c = k3[2, 2, 2]
    td = k3[:, 2, 2] / c
    th = k3[2, :, 2] / c
    tw = k3[2, 2, :] / c
    return td, th, tw, c


def _banded(taps, n):
    """B[i, j] = taps[i-j+2] for |i-j|<=2 (SAME zero padding via truncation)."""
    B = np.zeros((n, n), np.float64)
    for i in range(n):
        for j in range(max(0, i - 2), min(n, i + 3)):
            B[i, j] = taps[i - j + 2]
    return B


def _build_mats(weight):
    td, th, tw, c = _taps_from_weight(weight)
    bd64 = _banded(td, 64)
    BD = np.zeros((128, 128), np.float64)
    BD[:64, :64] = bd64
    BD[64:, 64:] = bd64  # block-diag: two volumes per pack, no cross mixing
    BW = _banded(tw, 128)
    BH = _banded(th, 128) * c  # fold the global scale into the last pass
    f16 = np.float16
    return BD.astype(f16), BW.astype(f16), BH.astype(f16)


def _build_program():
    import concourse.mybir as mybir
    from concourse import bacc, tile

    f32 = mybir.dt.float32
    f16 = mybir.dt.float16

    nc = bacc.Bacc(None)
    xin = nc.declare_dram_parameter("xin", [PACKS, 128, HW], f16, isOutput=False)
    bd = nc.declare_dram_parameter("bd", [128, 128], f16, isOutput=False)
    bw = nc.declare_dram_parameter("bw", [128, 128], f16, isOutput=False)
    bh = nc.declare_dram_parameter("bh", [128, 128], f16, isOutput=False)
    yout = nc.declare_dram_parameter("yout", [PACKS, 128, HW], f16, isOutput=True)

    with tile.TileContext(nc) as tc:
        with (
            tc.tile_pool(name="wts", bufs=1) as wts,
            tc.tile_pool(name="x16p", bufs=2) as x16p,
            tc.tile_pool(name="x3p", bufs=1) as x3p,
            tc.tile_pool(name="x5p", bufs=1) as x5p,
            tc.tile_pool(name="ps", bufs=4, space="PSUM") as psp,
            tc.tile_pool(name="st", bufs=4) as stp,
        ):
            BDt = wts.tile([128, 128], f16, tag="bd")
            BWt = wts.tile([128, 128], f16, tag="bw")
            BHt = wts.tile([128, 128], f16, tag="bh")
            nc.sync.dma_start(BDt[:], bd[:])
            nc.sync.dma_start(BWt[:], bw[:])
            nc.sync.dma_start(BHt[:], bh[:])

            def evac(t, dst, src):
                # alternate PSUM evacuations between DVE and ACT
                if t % 2 == 0:
                    nc.vector.tensor_copy(dst, src)
                else:
                    nc.scalar.copy(dst, src)

            for p in range(PACKS):
                x16 = x16p.tile([128, HW], f16, tag="x16")
                X3T = x3p.tile([128, HW], f16, tag="x3t")
                X5 = x5p.tile([128, HW], f16, tag="x5")

                # ---- load (already f16 on host) ----
                for ci in range(HW // LCHUNK):
                    sl = slice(ci * LCHUNK, (ci + 1) * LCHUNK)
                    nc.sync.dma_start(x16[:, sl], xin[p, :, sl])

                # ---- stage 1: conv-D (+ rotate w onto partitions) ----
                # lhsT = x16[:, h*128:(h+1)*128]  (partitions=vd, cols=w)
                # out[w, vd'] scattered -> X3T[w, vd'*128 + h]  (free = (vd, h))
                # evac iterates (vd, h8): strided PSUM reads, 16B SBUF runs
                X3Tv = X3T[:].rearrange("p (v h) -> p v h", h=128)
                for t in range(16):
                    ps = psp.tile([128, 1024], f32, tag="ps")
                    for hh in range(8):
                        h = 8 * t + hh
                        nc.tensor.matmul(
                            ps[:, hh * 128 : (hh + 1) * 128],
                            lhsT=x16[:, h * 128 : (h + 1) * 128],
                            rhs=BDt[:],
                        )
                    evac(
                        t,
                        X3Tv[:, :, 8 * t : 8 * t + 8],
                        ps[:].rearrange("p (h v) -> p v h", v=128),
                    )

                # ---- stage 2: conv-W (+ rotate h onto partitions) ----
                # lhsT = X3T[:, vd*128:(vd+1)*128]  (partitions=w, cols=h,
                # contiguous thanks to stage1's scattered store)
                # out[h, w'] -> X5[h, vd*128 + w']  (free = (vd, w))
                for t in range(16):
                    ps = psp.tile([128, 1024], f32, tag="ps")
                    for vv in range(8):
                        vd = 8 * t + vv
                        nc.tensor.matmul(
                            ps[:, vv * 128 : (vv + 1) * 128],
                            lhsT=X3T[:, vd * 128 : (vd + 1) * 128],
                            rhs=BWt[:],
                        )
                    evac(t, X5[:, t * 1024 : (t + 1) * 1024], ps[:])

                # ---- stage 3: conv-H (banded, stationary BH) ----
                # out[h', n] = sum_h BH[h, h'] X5[h, n]; cast f16, DMA out
                for t in range(16):
                    st = stp.tile([128, 1024], f16, tag="st")
                    ps = psp.tile([128, 1024], f32, tag="ps")
                    for u in range(2):
                        c = 2 * t + u
                        nc.tensor.matmul(
                            ps[:, u * 512 : (u + 1) * 512],
                            lhsT=BHt[:],
                            rhs=X5[:, c * 512 : (c + 1) * 512],
                        )
                    evac(t, st[:], ps[:])
                    nc.sync.dma_start(yout[p, :, t * 1024 : (t + 1) * 1024], st[:])
    nc.finalize()
    return nc


def _shard_inputs(x, weight):
    BD, BW, BH = _build_mats(weight)
    xs = np.asarray(x, np.float32).astype(np.float16).reshape(32, D, H, W)
    in_maps = []
    for k in range(N_CORES):
        core_x = xs[4 * k : 4 * k + 4].reshape(PACKS, 128, HW)
        in_maps.append(
            {
                "xin": np.ascontiguousarray(core_x),
                "bd": BD,
                "bw": BW,
                "bh": BH,
            }
        )
    return in_maps


def _unshard(results):
    # yout[p, h, vd*128 + w] holds out[vol=2p+v, d, h, w]  (f16 -> f32)
    vols = np.empty((32, D, H, W), np.float32)
    for k in range(N_CORES):
        y = results[k]["yout"].astype(np.float32).reshape(PACKS, H, 2, D, W)
        vols[4 * k : 4 * k + 4] = y.transpose(0, 2, 3, 1, 4).reshape(4, D, H, W)
    return vols.reshape(2, 16, D, H, W)


def kernel(x, weight):
    global _compiled
    from concourse.bass_utils import run_bass_kernel_spmd

    if _compiled is None:
        _compiled = _build_program()
    nc = _compiled
    in_maps = _shard_inputs(x, weight)
    res = run_bass_kernel_spmd(nc, in_maps, list(range(N_CORES)))
    return _unshard(res.results)
